# revision 34
# baseline (speedup 1.0000x reference)
"""FP8 dynamic-quantized linear (nn_FP8Linear) on 8 Trainium2 NeuronCores.

out = fp16((x_fp8 @ w_fp8.T) / (sx*sw)) + bias, with per-tensor dynamic
fp8-e4m3 quantization of x and weight (scale = FP8_MAX / amax).

Sharding: weight/bias split along out_features across 8 cores, x replicated.
Each core also receives a disjoint row-slice of x; per-core partial amaxes
are combined with one tiny AllReduce(max) so every core quantizes with the
global per-tensor scales (matching the reference exactly).

TRN fp8e4 (float8_e4m3) has max +-240 vs OCP e4m3fn's +-448, so the device
uses scale 224/amax == ref_scale/2: fp8 grids are self-similar under powers
of two, so device fp8 values are exactly half the reference's, and the
dequant multipliers (= 2x the reference's each) absorb the factor of 4.
"""

import numpy as np

import concourse.bacc as bacc
import concourse.bass as bass
import concourse.bass_isa as bass_isa
import concourse.mybir as mybir
import concourse.tile as tile
from concourse.bass_utils import run_bass_kernel_spmd

F16 = mybir.dt.float16
F32 = mybir.dt.float32
F8 = mybir.dt.float8e4

NCORES = 8
EPS = 1e-12
# device-side quantization scale numerator: ref uses 448 (e4m3fn max); we use
# 224 so quantized values stay within TRN e4m3's +-240 normal range.
DEV_FP8_MAX = 224.0


def build_kernel(M=4096, K=4096, NSH=512, SW=1024, double_row=False):
    """Build + compile the per-core bass program.

    M tokens, K in_features, NSH out_features per core, SW m-stripe width.
    double_row: use fp8 DoubleRow matmuls (~1.8x PE throughput but the PE's
    doubled-row accumulation path adds ~1e-4 relative noise); False uses
    normal-mode fp8 matmuls whose f32 accumulation is bit-faithful.
    """
    KCH = K // 128      # k-chunks of 128
    KB = K // 256       # k-blocks of 256 (DoubleRow contracts 256/pass)
    NSTRIPES = M // SW
    MCH = SW // 128     # m-chunks per stripe
    MS = M // NCORES    # rows of the per-core amax slice of x
    assert MS * K % 128 == 0

    nc = bacc.Bacc("TRN2", target_bir_lowering=False, debug=False,
                   num_devices=NCORES)
    x = nc.dram_tensor("x", [M, K], F16, kind="ExternalInput").ap()
    xs = nc.dram_tensor("xs", [MS, K], F16, kind="ExternalInput").ap()
    w = nc.dram_tensor("w", [NSH, K], F16, kind="ExternalInput").ap()
    bias = nc.dram_tensor("bias", [1, NSH], F16, kind="ExternalInput").ap()
    out = nc.dram_tensor("out", [M, NSH], F16, kind="ExternalOutput").ap()

    with tile.TileContext(nc) as tc:
        with (
            tc.tile_pool(name="const", bufs=1) as cpool,
            tc.tile_pool(name="redu", bufs=12) as rpool,
            tc.tile_pool(name="astg", bufs=2) as apool,
            tc.tile_pool(name="wstg", bufs=2) as wspool,
            tc.tile_pool(name="xstg", bufs=3) as xspool,
            tc.tile_pool(name="w8", bufs=KB) as w8pool,
            tc.tile_pool(name="x8", bufs=KB * NSTRIPES) as x8pool,
            tc.tile_pool(name="psum", bufs=4, space="PSUM") as ppool,
            tc.tile_pool(name="ot", bufs=4) as opool,
            tc.tile_pool(name="dram", bufs=2, space="DRAM") as dpool,
        ):
            # ---- bias broadcast to all partitions -------------------------
            bias_row = cpool.tile([1, NSH], F16, tag="bias_row")
            nc.gpsimd.dma_start(bias_row[:], bias[:])
            bias_b = cpool.tile([128, NSH], F16, tag="bias_b")
            nc.gpsimd.partition_broadcast(bias_b[:], bias_row[:])

            # ---- distributed amax: abs-max of local x row-slice + w shard -
            # x and w chunks are interleaved so DMA + DVE pipeline from t=0.
            def amax_chunks(dram_ap, total_elems, tag):
                flat = dram_ap.rearrange("a k -> (a k)").rearrange(
                    "(p f) -> p f", p=128)
                per_part = total_elems // 128
                nchunk = max(1, per_part // 2048)
                csz = per_part // nchunk
                return flat, nchunk, csz

            def combine(partials, tag):
                while len(partials) > 1:
                    nxt = []
                    for i in range(0, len(partials) - 1, 2):
                        m = rpool.tile([128, 1], F32, tag=f"pm_{tag}",
                                       name=f"pmc_{tag}_{len(partials)}_{i}")
                        nc.vector.tensor_tensor(
                            m[:], partials[i][:], partials[i + 1][:],
                            op=mybir.AluOpType.max)
                        nxt.append(m)
                    if len(partials) % 2:
                        nxt.append(partials[-1])
                    partials = nxt
                return partials[0]

            xflat, xnc, xcsz = amax_chunks(xs, MS * K, "x")
            wflat, wnc, wcsz = amax_chunks(w, NSH * K, "w")
            xparts, wparts = [], []
            amax_dmas = []
            for c in range(max(xnc, wnc)):
                for (flat, n, csz, parts, tag) in (
                        (xflat, xnc, xcsz, xparts, "x"),
                        (wflat, wnc, wcsz, wparts, "w")):
                    if c >= n:
                        continue
                    stg = apool.tile([128, csz], F16, tag="astg",
                                     name=f"astg_{tag}_{c}")
                    amax_dmas.append(nc.gpsimd.dma_start(
                        stg[:], flat[:, c * csz:(c + 1) * csz]))
                    pm = rpool.tile([128, 1], F32, tag=f"pm_{tag}",
                                    name=f"pm_{tag}_{c}")
                    nc.vector.tensor_reduce(
                        pm[:], stg[:], axis=mybir.AxisListType.X,
                        op=mybir.AluOpType.max, apply_absolute_value=True)
                    parts.append(pm)
            px = combine(xparts, "x")
            pw = combine(wparts, "w")

            amax2 = rpool.tile([128, 2], F32, tag="amax2")
            nc.vector.tensor_copy(amax2[:, 0:1], px[:])
            nc.vector.tensor_copy(amax2[:, 1:2], pw[:])
            amax2r = rpool.tile([128, 2], F32, tag="amax2r")
            nc.gpsimd.partition_all_reduce(
                amax2r[:], amax2[:], channels=128,
                reduce_op=bass_isa.ReduceOp.max)

            # ---- global amax via AllReduce(max) over the 8 cores ----------
            bin_ = dpool.tile([1, 2], F32)
            bout = dpool.tile([1, 2], F32)
            nc.gpsimd.dma_start(bin_[:], amax2r[0:1, :])
            nc.gpsimd.collective_compute(
                "AllReduce", mybir.AluOpType.max,
                replica_groups=[list(range(NCORES))],
                ins=[bin_.opt()], outs=[bout.opt()])
            g = rpool.tile([1, 2], F32, tag="g")
            nc.gpsimd.dma_start(g[:], bout[:])
            nc.vector.tensor_scalar_max(g[:], g[:], EPS)
            gb = rpool.tile([128, 2], F32, tag="gb")
            nc.gpsimd.partition_broadcast(gb[:], g[:])

            # scales: s = 224 * (1/amax), dequant r = 1/s
            u2 = rpool.tile([128, 2], F32, tag="u2")
            nc.vector.reciprocal(u2[:], gb[:])
            s2 = rpool.tile([128, 2], F32, tag="s2")
            nc.vector.tensor_scalar_mul(s2[:], u2[:], DEV_FP8_MAX)
            inv2 = rpool.tile([128, 2], F32, tag="inv2")
            nc.vector.reciprocal(inv2[:], s2[:])
            sx, sw = s2[:, 0:1], s2[:, 1:2]
            rx, rw = inv2[:, 0:1], inv2[:, 1:2]

            # ---- weight: transpose-load, quantize to w8 k-block tiles -----
            from concourse.bass import _add_dep_helper
            last_amax = amax_dmas[-1]
            w8 = []
            for kb in range(KB):
                w8.append(w8pool.tile([128, 2 * NSH], F8, tag="w8",
                          name=f"w8_{kb}"))
            # batched transposes: one DMA covers WB k-chunks via a 3D dest
            # (extra dims extend the partition dim: dest[p, c, n] = k-row
            # 128c+p), amortizing the per-transfer HWDGE overhead.
            WB = 4
            for b in range(KCH // WB):
                wstg = wspool.tile([128, WB, NSH], F16, tag="wstg")
                nc.sync.dma_start(
                    wstg[:], w[:, b * WB * 128:(b + 1) * WB * 128],
                    transpose=True)
                for j in range(WB):
                    c = b * WB + j
                    dst = w8[c // 2][:, (c % 2) * NSH:(c % 2 + 1) * NSH]
                    nc.scalar.activation(dst, wstg[:, j, :],
                                         mybir.ActivationFunctionType.Copy,
                                         scale=sw)

            # ---- x: per-stripe transpose-load + quantize, then matmuls ----
            # All x8 tiles are allocated up front; stripe 0's fp16 staging
            # borrows the (still empty) x8 tiles of the last stripes as
            # scratch, giving the DMA engines a deep dependency-free runway
            # while the amax AllReduce is still in flight. WAR tracking
            # orders the later stripes' real writes after the scratch reads.
            x8_all = []
            for s in range(NSTRIPES):
                x8_all.append([x8pool.tile([128, 2 * SW], F8, tag="x8",
                                           name=f"x8_{s}_{kb}")
                               for kb in range(KB)])
            XB = 4
            NB = KCH // XB
            for s in range(NSTRIPES):
                x8 = x8_all[s]
                for b in range(NB):
                    src = x[s * SW:(s + 1) * SW,
                            b * XB * 128:(b + 1) * XB * 128]
                    xstg = xspool.tile([128, XB, SW], F16, tag="xstg",
                                       name=f"xstg_{s}_{b}")[:]
                    nc.sync.dma_start(xstg, src, transpose=True)
                    for j in range(XB):
                        c = b * XB + j
                        dst = x8[c // 2][:, (c % 2) * SW:(c % 2 + 1) * SW]
                        if c % 2 == 0:
                            nc.vector.tensor_scalar(
                                dst, xstg[:, j, :], sx, None,
                                op0=mybir.AluOpType.mult)
                        else:
                            nc.scalar.activation(
                                dst, xstg[:, j, :],
                                mybir.ActivationFunctionType.Copy,
                                scale=sx)

                for mc in range(MCH):
                    ps = ppool.tile([128, NSH], F32, tag="ps")
                    if double_row:
                        for kb in range(KB):
                            lhsT = x8[kb].rearrange("p (i m) -> p i m", i=2)[
                                :, :, mc * 128:(mc + 1) * 128]
                            rhs = w8[kb].rearrange("p (i n) -> p i n", i=2)
                            nc.tensor.matmul(
                                ps[:], lhsT, rhs,
                                start=(kb == 0), stop=(kb == KB - 1),
                                perf_mode=mybir.MatmulPerfMode.DoubleRow)
                    else:
                        for kb in range(KB):
                            for i in range(2):
                                lhsT = x8[kb][:, i * SW + mc * 128:
                                              i * SW + (mc + 1) * 128]
                                rhs = w8[kb][:, i * NSH:(i + 1) * NSH]
                                nc.tensor.matmul(
                                    ps[:], lhsT, rhs,
                                    start=(kb == 0 and i == 0),
                                    stop=(kb == KB - 1 and i == 1))
                    ot = opool.tile([128, NSH], F16, tag="ot")
                    nc.vector.tensor_scalar(
                        ot[:], ps[:], rx, rw,
                        op0=mybir.AluOpType.mult, op1=mybir.AluOpType.mult)
                    nc.vector.tensor_tensor(ot[:], ot[:], bias_b[:],
                                            op=mybir.AluOpType.add)
                    m0 = s * SW + mc * 128
                    nc.gpsimd.dma_start(out[m0:m0 + 128, :], ot[:])

    nc.compile()
    return nc


DOUBLE_ROW = True

_CACHE = {}


def _get_kernel(M, K, NSH, SW, double_row=None):
    if double_row is None:
        double_row = DOUBLE_ROW
    key = (M, K, NSH, SW, double_row)
    if key not in _CACHE:
        _CACHE[key] = build_kernel(M, K, NSH, SW, double_row)
    return _CACHE[key]


def kernel(x, weight, bias):
    M, K = x.shape
    N = weight.shape[0]
    NSH = N // NCORES
    SW = 1024 if M % 1024 == 0 else M // 4
    nc = _get_kernel(M, K, NSH, SW)
    MS = M // NCORES

    x = np.asarray(x)
    weight = np.asarray(weight)
    bias = np.asarray(bias)
    in_maps = []
    for c in range(NCORES):
        in_maps.append({
            "x": x,
            "xs": np.ascontiguousarray(x[c * MS:(c + 1) * MS, :]),
            "w": np.ascontiguousarray(weight[c * NSH:(c + 1) * NSH, :]),
            "bias": np.ascontiguousarray(bias[c * NSH:(c + 1) * NSH]
                                         .reshape(1, NSH)),
        })
    res = run_bass_kernel_spmd(nc, in_maps, core_ids=list(range(NCORES)))
    return np.concatenate([res.results[c]["out"] for c in range(NCORES)],
                          axis=1)
